# revision 1
# baseline (speedup 1.0000x reference)
"""GQA attention kernel for Trainium2 (8 NeuronCores).

Problem: B=2, S=2048, D=2048, H=16 heads of DH=128, KV=4 kv heads, G=4
query heads per kv head.  Full (dense) attention, fp32 I/O.

Sharding: batch (2) x kv-head (4) = 8 cores, zero redundant FLOPs.
Each core computes, for its (batch b, kv head h):
    Q_g = x_b @ Wq[:, h,g]  (4 query heads), K = x_b @ Wk[:, h],
    V = x_b @ Wv[:, h], O_g = softmax(Q_g K^T / sqrt(DH)) V,
    y_partial = concat_g(O_g) @ Wo[h-rows, :]
Host sums the 4 kv-head partials per batch and adds bo.

On-chip strategy (all matmuls bf16 with fp32 PSUM accumulation):
 - host pre-transposes x (xT: [D, S]) and pre-casts weights to bf16
 - QT/KT computed head-transposed ([dh, s]) with W stationary, xT moving
 - S^T tiles ([k, q]) computed directly (KT-slice stationary, QT moving)
   so exp(S^T) lands in SBUF already transposed for the AV matmul:
   no P-transpose pass, no max-subtraction (scores are O(few), exp safe)
 - rowsum via ones-vector matmul accumulated alongside AV
 - softmax normalization folded into the PSUM->SBUF copy of O^T
   (multiply by DMA-broadcast 1/rowsum row)
 - out-proj: O^T stationary, Wo moving -> y natural, DMA'd straight
   from PSUM to DRAM.
"""

import sys

if "/opt/trn_rl_repo" not in sys.path:
    sys.path.insert(0, "/opt/trn_rl_repo")

import numpy as np
import ml_dtypes
from contextlib import ExitStack

B, S, D = 2, 2048, 2048
H, DH, GRP = 16, 128, 4
KV = H // GRP            # 4 kv heads
EH = GRP * DH            # 512 = query-head columns per kv head
SCALE = float(1.0 / np.sqrt(np.float32(DH)))
P = 128                  # partitions
NB = 512                 # matmul moving-dim block (one PSUM bank fp32)


def _emit(ctx, tc, aps, s=S, d=D, debug_taps=None):
    """Emit the per-core program. s, d parameterized for small-shape sim tests."""
    import concourse.bass as bass
    from concourse import mybir

    nc = tc.nc
    bf16 = mybir.dt.bfloat16
    f32 = mybir.dt.float32
    Exp = mybir.ActivationFunctionType.Exp
    Identity = mybir.ActivationFunctionType.Identity

    xt, wq, wk, wv, wo, bq, bk, bv, y = (
        aps["xt"], aps["wq"], aps["wk"], aps["wv"], aps["wo"],
        aps["bq"], aps["bk"], aps["bv"], aps["y"],
    )
    nt = s // P           # number of 128-tiles along s
    nd = d // P           # number of 128-tiles along d (contraction)
    nsb = s // NB         # number of 512-blocks along s
    ndb = d // NB         # number of 512-blocks along d (out columns)

    persist = ctx.enter_context(tc.tile_pool(name="persist", bufs=1))
    psum = ctx.enter_context(tc.tile_pool(name="psum", bufs=2, space="PSUM"))
    ptpool = ctx.enter_context(tc.tile_pool(name="ptp", bufs=2))
    rpool = ctx.enter_context(tc.tile_pool(name="rp", bufs=2))
    projpool = tc.tile_pool(name="projp", bufs=1)
    projp = projpool.__enter__()

    xt_sb = projp.tile([P, nd, s], bf16)
    wq_sb = projp.tile([P, nd, EH], bf16)
    wk_sb = projp.tile([P, nd, DH], bf16)
    wv_sb = projp.tile([P, nd, DH], bf16)
    wo_sb = persist.tile([P, GRP, d], bf16)
    qt_sb = persist.tile([P, GRP, s], bf16)
    kt_sb = persist.tile([P, s], bf16)
    v_sb = persist.tile([P, nt, DH], bf16)
    ot_sb = persist.tile([P, GRP, s], bf16)
    bq_sb = persist.tile([P, GRP], f32)
    bk_sb = persist.tile([P, 1], f32)
    bvb_sb = persist.tile([P, DH], f32)
    ones_sb = persist.tile([P, 1], bf16)

    nc.vector.memset(ones_sb, 1.0)

    # ---- loads ----
    xt_r = xt.rearrange("(t p) s -> p t s", p=P)
    wq_r = wq.rearrange("(t p) e -> p t e", p=P)
    wk_r = wk.rearrange("(t p) e -> p t e", p=P)
    wv_r = wv.rearrange("(t p) e -> p t e", p=P)
    wo_r = wo.rearrange("(g p) d -> p g d", p=P)
    for t in range(nd):
        nc.sync.dma_start(out=xt_sb[:, t, :], in_=xt_r[:, t, :])
        nc.sync.dma_start(out=wq_sb[:, t, :], in_=wq_r[:, t, :])
        nc.sync.dma_start(out=wk_sb[:, t, :], in_=wk_r[:, t, :])
        nc.sync.dma_start(out=wv_sb[:, t, :], in_=wv_r[:, t, :])
    for g in range(GRP):
        nc.sync.dma_start(out=wo_sb[:, g, :], in_=wo_r[:, g, :])
    nc.sync.dma_start(out=bq_sb, in_=bq.rearrange("(g p) -> p g", p=P))
    nc.sync.dma_start(out=bk_sb, in_=bk.rearrange("(p o) -> p o", o=1))
    # bv broadcast across partitions (varies along free dim of V)
    bv_bcast = bass.AP(tensor=bv.tensor, offset=bv.offset,
                       ap=[[0, P]] + list(bv.ap))
    nc.sync.dma_start(out=bvb_sb, in_=bv_bcast)

    # ---- projections ----
    # QT_g [dh, s] = (Wq_g)^T x^T, + bq*scale, scaled by 1/sqrt(DH)
    for g in range(GRP):
        for sb in range(nsb):
            ps = psum.tile([P, NB], f32, tag="mm")
            for t in range(nd):
                nc.tensor.matmul(
                    ps,
                    lhsT=wq_sb[:, t, g * DH:(g + 1) * DH],
                    rhs=xt_sb[:, t, sb * NB:(sb + 1) * NB],
                    start=(t == 0), stop=(t == nd - 1),
                )
            nc.scalar.activation(
                out=qt_sb[:, g, sb * NB:(sb + 1) * NB], in_=ps,
                func=Identity, bias=bq_sb[:, g:g + 1], scale=SCALE,
            )
    # KT [dh, s]
    for sb in range(nsb):
        ps = psum.tile([P, NB], f32, tag="mm")
        for t in range(nd):
            nc.tensor.matmul(
                ps, lhsT=wk_sb[:, t, :], rhs=xt_sb[:, t, sb * NB:(sb + 1) * NB],
                start=(t == 0), stop=(t == nd - 1),
            )
        nc.scalar.activation(
            out=kt_sb[:, sb * NB:(sb + 1) * NB], in_=ps,
            func=Identity, bias=bk_sb[:, 0:1], scale=1.0,
        )
    # V natural [k, dh] (xT stationary)
    for ki in range(nt):
        ps = psum.tile([P, NB], f32, tag="mm")
        for t in range(nd):
            nc.tensor.matmul(
                ps[:, 0:DH], lhsT=xt_sb[:, t, ki * P:(ki + 1) * P],
                rhs=wv_sb[:, t, :],
                start=(t == 0), stop=(t == nd - 1),
            )
        nc.vector.tensor_add(v_sb[:, ki, :], ps[:, 0:DH], bvb_sb)

    projpool.__exit__(None, None, None)

    # ---- attention ----
    # Software-pipelined: block n's S^T/exp interleave with block n-1's
    # AV + rowsum matmuls so PE never stalls waiting for ScalarE's exp.
    blocks = [(g, qb) for g in range(GRP) for qb in range(nsb)]

    def finish_block(prev):
        pg, pqb, ppt, pps_o, pps_r = prev
        pqsl = slice(pqb * NB, (pqb + 1) * NB)
        rrow = rpool.tile([1, NB], f32, tag="rrow")
        nc.vector.reciprocal(rrow, pps_r)
        rb = rpool.tile([P, NB], f32, tag="rb")
        nc.gpsimd.partition_broadcast(rb, rrow[0:1, :])
        nc.vector.tensor_mul(ot_sb[:, pg, pqsl], pps_o, rb)

    prev = None
    for g, qb in blocks:
        qsl = slice(qb * NB, (qb + 1) * NB)
        pt = ptpool.tile([P, nt, NB], bf16, tag="pt")
        ps_o = psum.tile([P, NB], f32, tag="o")
        ps_r = psum.tile([1, NB], f32, tag="r")
        for ki in range(nt):
            ps_s = psum.tile([P, NB], f32, tag="s")
            nc.tensor.matmul(
                ps_s, lhsT=kt_sb[:, ki * P:(ki + 1) * P],
                rhs=qt_sb[:, g, qsl], start=True, stop=True,
            )
            nc.scalar.activation(out=pt[:, ki, :], in_=ps_s, func=Exp)
            if prev is not None:
                _, _, ppt, pps_o, pps_r = prev
                nc.tensor.matmul(
                    pps_o, lhsT=v_sb[:, ki, :], rhs=ppt[:, ki, :],
                    start=(ki == 0), stop=(ki == nt - 1),
                )
                nc.tensor.matmul(
                    pps_r, lhsT=ones_sb[:, 0:1], rhs=ppt[:, ki, :],
                    start=(ki == 0), stop=(ki == nt - 1),
                )
        if prev is not None:
            finish_block(prev)
        prev = (g, qb, pt, ps_o, ps_r)
    # drain last block
    g, qb, pt, ps_o, ps_r = prev
    for ki in range(nt):
        nc.tensor.matmul(
            ps_o, lhsT=v_sb[:, ki, :], rhs=pt[:, ki, :],
            start=(ki == 0), stop=(ki == nt - 1),
        )
        nc.tensor.matmul(
            ps_r, lhsT=ones_sb[:, 0:1], rhs=pt[:, ki, :],
            start=(ki == 0), stop=(ki == nt - 1),
        )
    finish_block(prev)

    if debug_taps is not None:
        for name, t in [("qt", qt_sb), ("kt", kt_sb), ("v", v_sb),
                        ("ot", ot_sb), ("pt_last", None)]:
            if name in debug_taps and t is not None:
                nc.sync.dma_start(out=debug_taps[name], in_=t[:])

    # ---- out projection ----
    ypool = ctx.enter_context(tc.tile_pool(name="yp", bufs=2))
    for st in range(nt):
        for db in range(ndb):
            ps_y = psum.tile([P, NB], f32, tag="mm")
            for g in range(GRP):
                nc.tensor.matmul(
                    ps_y, lhsT=ot_sb[:, g, st * P:(st + 1) * P],
                    rhs=wo_sb[:, g, db * NB:(db + 1) * NB],
                    start=(g == 0), stop=(g == GRP - 1),
                )
            y_sb = ypool.tile([P, NB], f32, tag="y")
            if (st * ndb + db) % 2 == 0:
                nc.scalar.copy(y_sb, ps_y)
            else:
                nc.vector.tensor_copy(y_sb, ps_y)
            nc.sync.dma_start(
                out=y[st * P:(st + 1) * P, db * NB:(db + 1) * NB], in_=y_sb)


def build_program(s=S, d=D, debug=False):
    import concourse.tile as tile
    from concourse import bacc, mybir

    nc = bacc.Bacc("TRN2", target_bir_lowering=False, debug=False)
    bf16 = mybir.dt.bfloat16
    f32 = mybir.dt.float32
    aps = {
        "xt": nc.dram_tensor("xt", [d, s], bf16, kind="ExternalInput").ap(),
        "wq": nc.dram_tensor("wq", [d, EH], bf16, kind="ExternalInput").ap(),
        "wk": nc.dram_tensor("wk", [d, DH], bf16, kind="ExternalInput").ap(),
        "wv": nc.dram_tensor("wv", [d, DH], bf16, kind="ExternalInput").ap(),
        "wo": nc.dram_tensor("wo", [EH, d], bf16, kind="ExternalInput").ap(),
        "bq": nc.dram_tensor("bq", [EH], f32, kind="ExternalInput").ap(),
        "bk": nc.dram_tensor("bk", [DH], f32, kind="ExternalInput").ap(),
        "bv": nc.dram_tensor("bv", [DH], f32, kind="ExternalInput").ap(),
        "y": nc.dram_tensor("y", [s, d], f32, kind="ExternalOutput").ap(),
    }
    debug_taps = None
    if debug:
        nt = s // P
        debug_taps = {
            "qt": nc.dram_tensor("dbg_qt", [P, GRP, s], bf16, kind="ExternalOutput").ap(),
            "kt": nc.dram_tensor("dbg_kt", [P, s], bf16, kind="ExternalOutput").ap(),
            "v": nc.dram_tensor("dbg_v", [P, nt, DH], bf16, kind="ExternalOutput").ap(),
            "ot": nc.dram_tensor("dbg_ot", [P, GRP, s], bf16, kind="ExternalOutput").ap(),
        }
    with tile.TileContext(nc) as tc:
        with ExitStack() as ctx:
            _emit(ctx, tc, aps, s=s, d=d, debug_taps=debug_taps)
    nc.compile()
    return nc


def make_in_maps(x, Wq, bq, Wk, bk, Wv, bv, Wo, bo):
    bf = ml_dtypes.bfloat16
    in_maps = []
    for b in range(B):
        xt_b = x[b].T.astype(bf)  # [D, S] contiguous
        for h in range(KV):
            in_maps.append({
                "xt": xt_b,
                "wq": Wq[:, h * EH:(h + 1) * EH].astype(bf),
                "wk": Wk[:, h * DH:(h + 1) * DH].astype(bf),
                "wv": Wv[:, h * DH:(h + 1) * DH].astype(bf),
                "wo": np.ascontiguousarray(Wo[h * EH:(h + 1) * EH, :]).astype(bf),
                "bq": (bq[h * EH:(h + 1) * EH] * SCALE).astype(np.float32),
                "bk": np.ascontiguousarray(bk[h * DH:(h + 1) * DH]).astype(np.float32),
                "bv": np.ascontiguousarray(bv[h * DH:(h + 1) * DH]).astype(np.float32),
            })
    return in_maps


_PROG = None


def _get_program():
    global _PROG
    if _PROG is None:
        _PROG = build_program()
    return _PROG


def run_cores(in_maps, trace=False, **kw):
    from concourse.bass_utils import run_bass_kernel_spmd
    nc = _get_program()
    return run_bass_kernel_spmd(nc, in_maps, list(range(8)), trace=trace, **kw)


def kernel(**inputs):
    x = np.asarray(inputs["x"], dtype=np.float32)
    Wq = np.asarray(inputs["Wq"], dtype=np.float32)
    bq = np.asarray(inputs["bq"], dtype=np.float32)
    Wk = np.asarray(inputs["Wk"], dtype=np.float32)
    bk = np.asarray(inputs["bk"], dtype=np.float32)
    Wv = np.asarray(inputs["Wv"], dtype=np.float32)
    bv = np.asarray(inputs["bv"], dtype=np.float32)
    Wo = np.asarray(inputs["Wo"], dtype=np.float32)
    bo = np.asarray(inputs["bo"], dtype=np.float32)

    in_maps = make_in_maps(x, Wq, bq, Wk, bk, Wv, bv, Wo, bo)
    res = run_cores(in_maps)
    out = np.empty((B, S, D), dtype=np.float32)
    for b in range(B):
        acc = res.results[b * KV]["y"].astype(np.float32)
        for h in range(1, KV):
            acc = acc + res.results[b * KV + h]["y"]
        out[b] = acc + bo[None, :]
    return out



# revision 31
# speedup vs baseline: 1.5106x; 1.5106x over previous
"""GQA attention kernel for Trainium2 (8 NeuronCores).

Problem: B=2, S=2048, D=2048, H=16 heads of DH=128, KV=4 kv heads, G=4
query heads per kv head.  Full (dense) attention, fp32 I/O.

Sharding: batch (2) x kv-head (4) = 8 cores, zero redundant FLOPs.
Each core computes, for its (batch b, kv head h):
    Q_g = x_b @ Wq[:, h,g]  (4 query heads), K = x_b @ Wk[:, h],
    V = x_b @ Wv[:, h], O_g = softmax(Q_g K^T / sqrt(DH)) V,
    y_partial = concat_g(O_g) @ Wo[h-rows, :]
Host sums the 4 kv-head partials per batch and adds bo.

On-chip schedule (all matmuls bf16, fp32 PSUM):
 - projections emitted t-outer in waves (K+Q0, Q1+Q2, Q3+V) so the PE
   consumes each xt contraction tile as its DMA lands; 8 PSUM banks hold
   the wave's accumulators.
 - attention runs in 4 rounds (one per 512-wide q block), 4 kv-grouped
   query heads per round.  Scores are computed transposed ([k, q]) so
   exp(S^T) lands ready for the AV matmul; exp is batched 2 PSUM banks
   (1024 elems) per ScalarE instruction.
 - AV + rowsum run one head-block behind scores (software pipeline).
 - rowsum: DVE in-place pairwise tree on exp(S^T) (16->2 tiles, 2x bf16
   mode) + 2 ones-matmuls, instead of 16 PE ones-matmuls.
 - out-projection of round r-1 is interleaved into round r's PE stream
   (one block per scores-pair slot) so the PE stays busy while ScalarE
   works through the exps; PSUM->SBUF copies alternate ScalarE/DVE.
"""

import sys

if "/opt/trn_rl_repo" not in sys.path:
    sys.path.insert(0, "/opt/trn_rl_repo")

import numpy as np
import ml_dtypes
from contextlib import ExitStack

B, S, D = 2, 2048, 2048
H, DH, GRP = 16, 128, 4
KV = H // GRP            # 4 kv heads
EH = GRP * DH            # 512 = query-head columns per kv head
SCALE = float(1.0 / np.sqrt(np.float32(DH)))
P = 128                  # partitions
DBG_NO_EARLY_TREE = True
DBG_SIMPLE_LOADS = True
NB = 512                 # matmul moving-dim block (one PSUM bank fp32)


def _emit(ctx, tc, aps, s=S, d=D):
    import concourse.bass as bass
    from concourse import mybir

    nc = tc.nc
    bf16 = mybir.dt.bfloat16
    f32 = mybir.dt.float32
    Exp = mybir.ActivationFunctionType.Exp
    Identity = mybir.ActivationFunctionType.Identity

    xt, wq, wk, wv, wo, bq, bk, bv, y = (
        aps["xt"], aps["wq"], aps["wk"], aps["wv"], aps["wo"],
        aps["bq"], aps["bk"], aps["bv"], aps["y"],
    )
    nt = s // P           # 128-tiles along s
    nd = d // P           # 128-tiles along d (contraction)
    nsb = s // NB         # 512-blocks along s (q rounds)
    ndb = d // NB         # 512-blocks along d (out columns)
    npair = nt // 2       # exp pairs per block

    persist = ctx.enter_context(tc.tile_pool(name="persist", bufs=1))

    xt_sb = persist.tile([P, nd, s], bf16)
    wq_sb = persist.tile([P, nd, EH], bf16)
    wk_sb = persist.tile([P, nd, DH], bf16)
    wv_sb = persist.tile([P, nd, DH], bf16)
    wo_sb = persist.tile([P, GRP, d], bf16)
    qt_sb = persist.tile([P, GRP, s], bf16)
    kt_sb = persist.tile([P, s], bf16)
    v_sb = persist.tile([P, nt, DH], bf16)
    ot_sb = persist.tile([P, GRP, s], bf16)
    bq_sb = persist.tile([P, GRP], f32)
    bk_sb = persist.tile([P, 1], f32)
    bvb_sb = persist.tile([P, DH], f32)
    ones_sb = persist.tile([P, 1], bf16)

    nc.vector.memset(ones_sb, 1.0)

    # ---- loads (in consumption order; xt tiles pace wave A) ----
    xt_r = xt.rearrange("(t p) s -> p t s", p=P)
    wq_r = wq.rearrange("(t p) e -> p t e", p=P)
    wk_r = wk.rearrange("(t p) e -> p t e", p=P)
    wv_r = wv.rearrange("(t p) e -> p t e", p=P)
    wo_r = wo.rearrange("(g p) d -> p g d", p=P)
    bv_bcast = bass.AP(tensor=bv.tensor, offset=bv.offset,
                       ap=[[0, P]] + list(bv.ap))
    if nd >= 16 and not DBG_SIMPLE_LOADS:
        nc.sync.dma_start(out=wk_sb[:, 0:8, :], in_=wk_r[:, 0:8, :])
        nc.sync.dma_start(out=xt_sb[:, 0, :], in_=xt_r[:, 0, :])
        nc.sync.dma_start(out=wq_sb[:, 0, :], in_=wq_r[:, 0, :])
        nc.sync.dma_start(out=xt_sb[:, 1, :], in_=xt_r[:, 1, :])
        nc.sync.dma_start(out=wq_sb[:, 1, :], in_=wq_r[:, 1, :])
        nc.sync.dma_start(out=bq_sb, in_=bq.rearrange("(g p) -> p g", p=P))
        nc.sync.dma_start(out=bk_sb, in_=bk.rearrange("(p o) -> p o", o=1))
        nc.sync.dma_start(out=bvb_sb, in_=bv_bcast)
        nc.sync.dma_start(out=wk_sb[:, 8:, :], in_=wk_r[:, 8:, :])
        for t in range(2, nd):
            nc.sync.dma_start(out=wq_sb[:, t, :], in_=wq_r[:, t, :])
            nc.sync.dma_start(out=xt_sb[:, t, :], in_=xt_r[:, t, :])
        nc.sync.dma_start(out=wv_sb, in_=wv_r)
    else:
        nc.sync.dma_start(out=wk_sb, in_=wk_r)
        nc.sync.dma_start(out=wv_sb, in_=wv_r)
        nc.sync.dma_start(out=bq_sb, in_=bq.rearrange("(g p) -> p g", p=P))
        nc.sync.dma_start(out=bk_sb, in_=bk.rearrange("(p o) -> p o", o=1))
        nc.sync.dma_start(out=bvb_sb, in_=bv_bcast)
        for t in range(nd):
            nc.sync.dma_start(out=xt_sb[:, t, :], in_=xt_r[:, t, :])
            nc.sync.dma_start(out=wq_sb[:, t, :], in_=wq_r[:, t, :])
    for g in range(GRP):
        nc.sync.dma_start(out=wo_sb[:, g, :], in_=wo_r[:, g, :])

    # ---- projections: t-outer waves over 8 PSUM accumulators ----
    with tc.tile_pool(name="pj", bufs=8, space="PSUM") as pj:

        def q_epilogue(g, qb, ps, dve=False):
            out = qt_sb[:, g, qb * NB:(qb + 1) * NB]
            if dve:
                nc.vector.tensor_scalar(
                    out=out, in0=ps, scalar1=SCALE, scalar2=bq_sb[:, g:g + 1],
                    op0=mybir.AluOpType.mult, op1=mybir.AluOpType.add)
            else:
                nc.scalar.activation(
                    out=out, in_=ps,
                    func=Identity, bias=bq_sb[:, g:g + 1], scale=SCALE,
                )

        # Waves of up to 8 single-bank accumulators, t-outer so the PE
        # consumes each xt contraction tile as its DMA lands.  Each
        # accumulator owns a full PSUM bank: interleaving accumulation
        # groups within one bank corrupts results.  Epilogues are emitted
        # right after each accumulator's stop matmul and split ScalarE/DVE
        # so banks free quickly for the next wave / attention.
        items = ([("k", sb) for sb in range(nsb)]
                 + [("q", qb, g) for qb in range(nsb) for g in range(GRP)]
                 + [("v", ki) for ki in range(nt)])
        waves = [items[i:i + 8] for i in range(0, len(items), 8)]
        for wave in waves:
            ps_w = [pj.tile([P, NB], f32, tag="pj", name=f"ps_{''.join(map(str, it))}")
                    for it in wave]
            for t in range(nd):
                fl = dict(start=(t == 0), stop=(t == nd - 1))
                for ps, it in zip(ps_w, wave):
                    if it[0] == "k":
                        sb = it[1]
                        nc.tensor.matmul(
                            ps, lhsT=wk_sb[:, t, :],
                            rhs=xt_sb[:, t, sb * NB:(sb + 1) * NB], **fl)
                    elif it[0] == "q":
                        _, qb, g = it
                        nc.tensor.matmul(
                            ps, lhsT=wq_sb[:, t, g * DH:(g + 1) * DH],
                            rhs=xt_sb[:, t, qb * NB:(qb + 1) * NB], **fl)
                    else:
                        ki = it[1]
                        nc.tensor.matmul(
                            ps[:, 0:DH],
                            lhsT=xt_sb[:, t, ki * P:(ki + 1) * P],
                            rhs=wv_sb[:, t, :], **fl)
                    if t == nd - 1:
                        if it[0] == "k":
                            nc.scalar.activation(
                                out=kt_sb[:, it[1] * NB:(it[1] + 1) * NB],
                                in_=ps, func=Identity,
                                bias=bk_sb[:, 0:1], scale=1.0)
                        elif it[0] == "q":
                            q_epilogue(it[2], it[1], ps, dve=(it[2] % 2 == 1))
                        else:
                            nc.vector.tensor_add(
                                v_sb[:, it[1], :], ps[:, 0:DH], bvb_sb)

    # ---- attention + interleaved out-projection ----
    ps2 = ctx.enter_context(tc.tile_pool(name="ps2", bufs=2, space="PSUM"))
    ps1 = ctx.enter_context(tc.tile_pool(name="ps1", bufs=1, space="PSUM"))
    ptpool = ctx.enter_context(tc.tile_pool(name="ptp", bufs=2))
    rpool = ctx.enter_context(tc.tile_pool(name="rp", bufs=2))
    ypool = ctx.enter_context(tc.tile_pool(name="yp", bufs=6))

    def emit_outproj(st, db, rotate=False, scalar_copy=False):
        """One out-projection block: y[st,db] = sum_g ot[g, st] @ wo[g, db]."""
        if rotate:
            # drain phase: the scores banks are free; rotate over 3 banks
            ps_y = ps2.tile([P, NB], f32, tag="s", name="ps_ys")
        else:
            ps_y = ps1.tile([P, NB], f32, tag="y")
        for g in range(GRP):
            nc.tensor.matmul(
                ps_y, lhsT=ot_sb[:, g, st * P:(st + 1) * P],
                rhs=wo_sb[:, g, db * NB:(db + 1) * NB],
                start=(g == 0), stop=(g == GRP - 1))
        y_sb = ypool.tile([P, NB], f32, tag="y")
        if scalar_copy:
            nc.scalar.copy(y_sb, ps_y)
        else:
            nc.vector.tensor_copy(y_sb, ps_y)
        nc.sync.dma_start(
            out=y[st * P:(st + 1) * P, db * NB:(db + 1) * NB], in_=y_sb)

    def emit_tree(prev, w):
        """One level of the in-place pairwise rowsum tree over the nt axis.
        Level w=8 only reads exp tiles the prior block's AV already consumed,
        so levels can be emitted inside the next block's pair loop."""
        ppt = prev[2]
        nc.vector.tensor_add(
            ppt[:, 0:w, :], ppt[:, 0:w, :], ppt[:, w:2 * w, :])

    def emit_epilogue(prev):
        """Rowsum finish + normalize for a finished block (runs off PE)."""
        pg, pqb, ppt, pps_o = prev
        qsl = slice(pqb * NB, (pqb + 1) * NB)
        if DBG_NO_EARLY_TREE:
            for w in (8, 4):
                if w * 2 <= (S // P):
                    emit_tree(prev, w)
        emit_tree(prev, 2)
        ps_r = ps1.tile([1, NB], f32, tag="r")
        for j in range(2):
            nc.tensor.matmul(
                ps_r, lhsT=ones_sb[:, 0:1], rhs=ppt[:, j, :],
                start=(j == 0), stop=(j == 1))
        rrow = rpool.tile([1, NB], f32, tag="rrow")
        nc.vector.reciprocal(rrow, ps_r)
        rb = rpool.tile([P, NB], f32, tag="rb")
        nc.gpsimd.partition_broadcast(rb, rrow[0:1, :])
        nc.vector.tensor_mul(ot_sb[:, pg, qsl], pps_o, rb)

    prev = None          # (g, qb, pt, ps_o) one block behind
    fillers = []         # pending out-proj blocks to interleave
    for qb in range(nsb):
        for g in range(GRP):
            qsl = slice(qb * NB, (qb + 1) * NB)
            pt = ptpool.tile([P, nt, NB], bf16, tag="pt")
            ps_o = ps2.tile([P, NB], f32, tag="o")
            for j in range(npair):
                ps_s = ps2.tile([P, 2, NB], f32, tag="s")
                for h in range(2):
                    ki = 2 * j + h
                    nc.tensor.matmul(
                        ps_s[:, h, :], lhsT=kt_sb[:, ki * P:(ki + 1) * P],
                        rhs=qt_sb[:, g, qsl], start=True, stop=True)
                nc.scalar.activation(
                    out=pt[:, 2 * j:2 * j + 2, :], in_=ps_s, func=Exp)
                if prev is not None:
                    _, _, ppt, pps_o = prev
                    for h in range(2):
                        ki = 2 * j + h
                        nc.tensor.matmul(
                            pps_o, lhsT=v_sb[:, ki, :], rhs=ppt[:, ki, :],
                            start=(ki == 0), stop=(ki == nt - 1))
                    if j == 3 and not DBG_NO_EARLY_TREE:
                        emit_tree(prev, 8)
                    elif j == 5 and not DBG_NO_EARLY_TREE:
                        emit_tree(prev, 4)
                # pop 14 fillers per round (leave a few to cover the
                # drain's epilogue-chain latency)
                if j % 2 == 1 and not (j == npair - 1 and g >= 2) and fillers:
                    emit_outproj(*fillers.pop(0))
            if prev is not None:
                emit_epilogue(prev)
            prev = (g, qb, pt, ps_o)
        # out-proj for this round's q-tiles becomes next round's filler
        fillers.extend(
            (qb * GRP + i, db) for i in range(GRP) for db in range(ndb))

    # drain: AV + epilogue of the last block; held-back out-proj blocks of
    # earlier rounds cover the epilogue chain latency, then the last round's.
    g, qb, pt, ps_o = prev
    early = [f for f in fillers if f[0] < (nsb - 1) * GRP]
    late = [f for f in fillers if f[0] >= (nsb - 1) * GRP]
    for ki in range(nt):
        nc.tensor.matmul(
            ps_o, lhsT=v_sb[:, ki, :], rhs=pt[:, ki, :],
            start=(ki == 0), stop=(ki == nt - 1))
        if ki == 7 and not DBG_NO_EARLY_TREE:
            emit_tree(prev, 8)
        elif ki == 11 and not DBG_NO_EARLY_TREE:
            emit_tree(prev, 4)
    # early fillers (previous rounds' q-tiles) keep the PE busy while the
    # last block's rowsum/normalize chain runs; they must be emitted before
    # the final epilogue so they don't wait on its ot write.
    # ScalarE is idle during the drain: route copies there so DVE's queue
    # stays clear for the rowsum/normalize chain, then alternate.
    for i, (st, db) in enumerate(early):
        emit_outproj(st, db, rotate=(i % 2 == 1), scalar_copy=True)
    emit_epilogue(prev)
    for i, (st, db) in enumerate(late):
        emit_outproj(st, db, rotate=(i % 2 == 1), scalar_copy=(i % 2 == 0))


def build_program(s=S, d=D):
    import concourse.tile as tile
    from concourse import bacc, mybir

    nc = bacc.Bacc("TRN2", target_bir_lowering=False, debug=False)
    bf16 = mybir.dt.bfloat16
    f32 = mybir.dt.float32
    aps = {
        "xt": nc.dram_tensor("xt", [d, s], bf16, kind="ExternalInput").ap(),
        "wq": nc.dram_tensor("wq", [d, EH], bf16, kind="ExternalInput").ap(),
        "wk": nc.dram_tensor("wk", [d, DH], bf16, kind="ExternalInput").ap(),
        "wv": nc.dram_tensor("wv", [d, DH], bf16, kind="ExternalInput").ap(),
        "wo": nc.dram_tensor("wo", [EH, d], bf16, kind="ExternalInput").ap(),
        "bq": nc.dram_tensor("bq", [EH], f32, kind="ExternalInput").ap(),
        "bk": nc.dram_tensor("bk", [DH], f32, kind="ExternalInput").ap(),
        "bv": nc.dram_tensor("bv", [DH], f32, kind="ExternalInput").ap(),
        "y": nc.dram_tensor("y", [s, d], f32, kind="ExternalOutput").ap(),
    }
    with tile.TileContext(nc) as tc:
        with ExitStack() as ctx:
            _emit(ctx, tc, aps, s=s, d=d)
    nc.compile()
    return nc


def make_in_maps(x, Wq, bq, Wk, bk, Wv, bv, Wo, bo):
    bf = ml_dtypes.bfloat16
    in_maps = []
    for b in range(B):
        xt_b = x[b].T.astype(bf)  # [D, S] contiguous
        for h in range(KV):
            in_maps.append({
                "xt": xt_b,
                "wq": Wq[:, h * EH:(h + 1) * EH].astype(bf),
                "wk": Wk[:, h * DH:(h + 1) * DH].astype(bf),
                "wv": Wv[:, h * DH:(h + 1) * DH].astype(bf),
                "wo": np.ascontiguousarray(Wo[h * EH:(h + 1) * EH, :]).astype(bf),
                "bq": (bq[h * EH:(h + 1) * EH] * SCALE).astype(np.float32),
                "bk": np.ascontiguousarray(bk[h * DH:(h + 1) * DH]).astype(np.float32),
                "bv": np.ascontiguousarray(bv[h * DH:(h + 1) * DH]).astype(np.float32),
            })
    return in_maps


_PROG = None


def _get_program():
    global _PROG
    if _PROG is None:
        _PROG = build_program()
    return _PROG


def run_cores(in_maps, trace=False, **kw):
    from concourse.bass_utils import run_bass_kernel_spmd
    nc = _get_program()
    return run_bass_kernel_spmd(nc, in_maps, list(range(8)), trace=trace, **kw)


def kernel(**inputs):
    x = np.asarray(inputs["x"], dtype=np.float32)
    Wq = np.asarray(inputs["Wq"], dtype=np.float32)
    bq = np.asarray(inputs["bq"], dtype=np.float32)
    Wk = np.asarray(inputs["Wk"], dtype=np.float32)
    bk = np.asarray(inputs["bk"], dtype=np.float32)
    Wv = np.asarray(inputs["Wv"], dtype=np.float32)
    bv = np.asarray(inputs["bv"], dtype=np.float32)
    Wo = np.asarray(inputs["Wo"], dtype=np.float32)
    bo = np.asarray(inputs["bo"], dtype=np.float32)

    in_maps = make_in_maps(x, Wq, bq, Wk, bk, Wv, bv, Wo, bo)
    res = run_cores(in_maps)
    out = np.empty((B, S, D), dtype=np.float32)
    for b in range(B):
        acc = res.results[b * KV]["y"].astype(np.float32)
        for h in range(1, KV):
            acc = acc + res.results[b * KV + h]["y"]
        out[b] = acc + bo[None, :]
    return out
